# revision 12
# baseline (speedup 1.0000x reference)
"""SATD loss kernel for Trainium2: sum |H @ (original - pred)|.

Full inputs: original, pred [2, 8192, 64, 64] f32. H is the 64x64
Sylvester Hadamard matrix applied along axis -2 of each 64x64 block.

Strategy (8-way data parallel, 2048 blocks per core):
  - Host: diff = original - pred quantized to e4m3 (single rounding,
    ~3.5e-4 rel err on the scalar) -> halves HBM traffic (8.4 MB/core)
    and PE streaming vs shipping both operands.
  - Device: t = kron(I2, H64) @ x, one fp8 matmul per 512 moving cols
    (PSUM-bank cap). Back-to-back matmuls run at ~216 ns (LDWEIGHTS
    overlaps the previous matmul), so the PE is not the wall.
  - The wall is the PSUM drain: PSUM has one read port per engine,
    every reduce-capable op runs at 1 elem/cycle/partition, GPSIMD has
    no PSUM port, and the BIR verifier forbids two PSUM operands in
    one instruction. So the 65536 t-columns per core leave PSUM via
    VectorE tensor_reduce(abs+add) and ScalarE activation(Abs,
    accum_out) in parallel on different banks, 32 tiles each.
  - Each engine has its OWN double-buffered [128, 1024] PSUM pool
    (4 tiles = all 8 banks). Separate pools avoid head-of-line
    blocking on the in-order PE queue: a slow drain of one engine
    never blocks refills of the other engine's bank pair.
  - Per-tile partials land in one [128, 64] accumulator, DMA'd out
    once; the host does the final (tiny) summation in f64.
"""

from contextlib import ExitStack

import ml_dtypes
import numpy as np

import concourse.bass as bass
import concourse.tile as tile
from concourse import bacc, mybir
from concourse.bass_utils import run_bass_kernel_spmd

N_CORES = 8
N = 64                        # Hadamard block size
BLOCKS_TOTAL = 2 * 8192       # 16384 blocks of [64, 64]
BLOCKS_PER_CORE = BLOCKS_TOTAL // N_CORES   # 2048
GROUP_COLS = 1024             # psum tile free size (2 banks)
GROUPS = 64                   # psum tiles per core
GROUPS_PER_TILE = 8           # SBUF x-tile holds 8 groups (8 KiB/part)
ND = 33                       # tiles drained by VectorE
NA = GROUPS - ND              # tiles drained by ScalarE (1397 ns each)
TILES = GROUPS // GROUPS_PER_TILE           # 8
MM_N = 512                    # matmul moving cols (one PSUM bank out)

F32 = mybir.dt.float32
IN_DT = mybir.dt.float8e4
IN_NP = ml_dtypes.float8_e4m3


def _hadamard(n: int) -> np.ndarray:
    H = np.array([[1.0]], dtype=np.float32)
    while H.shape[0] < n:
        H = np.block([[H, H], [H, -H]])
    return H.astype(np.float32)


def _weights() -> np.ndarray:
    # lhsT for out = W @ rhs is W.T; kron(I2, H64) is symmetric.
    return np.kron(np.eye(2, dtype=np.float32),
                   _hadamard(N)).astype(IN_NP)  # [128, 128]


def _build_program() -> bacc.Bacc:
    nc = bacc.Bacc("TRN2", target_bir_lowering=False, debug=False,
                   num_devices=N_CORES)
    x = nc.dram_tensor("x", [TILES, 128, GROUPS_PER_TILE * GROUP_COLS],
                       IN_DT, kind="ExternalInput").ap()
    w = nc.dram_tensor("w", [128, 128], IN_DT, kind="ExternalInput").ap()
    out = nc.dram_tensor("out", [128, GROUPS], F32,
                         kind="ExternalOutput").ap()

    with tile.TileContext(nc) as tc, ExitStack() as ctx:
        wpool = ctx.enter_context(tc.tile_pool(name="w", bufs=1))
        xpool = ctx.enter_context(tc.tile_pool(name="x", bufs=3))
        psumd = ctx.enter_context(tc.tile_pool(name="psumd", bufs=2,
                                               space="PSUM"))
        psuma = ctx.enter_context(tc.tile_pool(name="psuma", bufs=2,
                                               space="PSUM"))
        accpool = ctx.enter_context(tc.tile_pool(name="acc", bufs=1))
        scrpool = ctx.enter_context(tc.tile_pool(name="scr", bufs=2))

        wt = wpool.tile([128, 128], IN_DT)

        acc = accpool.tile([128, GROUPS], F32, tag="acc")

        # Bresenham interleave of ND VectorE / NA ScalarE drains
        sched = []
        nd, na, err = ND, NA, 0
        while nd or na:
            if na == 0 or (nd > 0 and err <= 0):
                sched.append("D"); nd -= 1; err += NA
            else:
                sched.append("A"); na -= 1; err -= ND
        gi = 0
        di = 0
        ai = 0
        for t in range(TILES):
            xt = xpool.tile([128, GROUPS_PER_TILE * GROUP_COLS], IN_DT)
            if t == 0:
                # First x chunk before w: the first matmul needs both and
                # the 128 KiB chunk is the long pole; escalating chunk
                # sizes keep descriptor generation off the critical path.
                nc.sync.dma_start(xt[:, 0:1024], x[t, :, 0:1024])
                nc.scalar.dma_start(wt[:], w[:])
                nc.sync.dma_start(xt[:, 1024:4096], x[t, :, 1024:4096])
                nc.sync.dma_start(xt[:, 4096:8192], x[t, :, 4096:8192])
            else:
                nc.sync.dma_start(xt[:], x[t])
            for g in range(GROUPS_PER_TILE):
                pt = (psumd if sched[gi] == "D" else psuma).tile(
                    [128, GROUP_COLS], F32)
                for q in range(GROUP_COLS // MM_N):
                    lo = g * GROUP_COLS + q * MM_N
                    nc.tensor.matmul(
                        pt[:, q * MM_N:(q + 1) * MM_N], wt[:],
                        xt[:, lo:lo + MM_N], start=True, stop=True)
                if sched[gi] == "D":
                    nc.vector.tensor_reduce(
                        acc[:, di:di + 1], pt[:],
                        axis=mybir.AxisListType.X, op=mybir.AluOpType.add,
                        apply_absolute_value=True)
                    di += 1
                else:
                    st = scrpool.tile([128, GROUP_COLS], F32)
                    nc.scalar.activation(
                        st[:], pt[:], mybir.ActivationFunctionType.Abs,
                        accum_out=acc[:, ND + ai:ND + ai + 1])
                    ai += 1
                gi += 1

        nc.sync.dma_start(out[:], acc[:])

    nc.compile()
    return nc


def _repack_core(shard: np.ndarray) -> np.ndarray:
    """[2048, 64, 64] e4m3 -> [TILES, 128, 8192]: partitions (m, j64),
    free (blk, k) within each SBUF tile."""
    v = shard.reshape(TILES, 2, 128, N, N)
    return v.transpose(0, 1, 3, 2, 4).reshape(
        TILES, 128, GROUPS_PER_TILE * GROUP_COLS)


_NC = None


def _get_program() -> bacc.Bacc:
    global _NC
    if _NC is None:
        _NC = _build_program()
    return _NC


def _run(original: np.ndarray, pred: np.ndarray, **spmd_kwargs):
    a = np.asarray(original, dtype=np.float32).reshape(BLOCKS_TOTAL, N, N)
    b = np.asarray(pred, dtype=np.float32).reshape(BLOCKS_TOTAL, N, N)
    diff = (a - b).astype(IN_NP)
    wnp = _weights()
    in_maps = []
    for i in range(N_CORES):
        sl = slice(i * BLOCKS_PER_CORE, (i + 1) * BLOCKS_PER_CORE)
        in_maps.append({"x": _repack_core(diff[sl]), "w": wnp})
    nc = _get_program()
    r = run_bass_kernel_spmd(nc, in_maps, list(range(N_CORES)),
                             **spmd_kwargs)
    total = 0.0
    for i in range(N_CORES):
        total += r.results[i]["out"].astype(np.float64).sum()
    return np.float32(total), r


def kernel(original: np.ndarray, pred: np.ndarray) -> np.ndarray:
    val, _ = _run(original, pred)
    return np.array(val, dtype=np.float32)


# revision 13
# speedup vs baseline: 1.1726x; 1.1726x over previous
"""SATD loss kernel for Trainium2: sum |H @ (original - pred)|.

Full inputs: original, pred [2, 8192, 64, 64] f32. H is the 64x64
Sylvester Hadamard matrix applied along axis -2 of each 64x64 block.

Strategy (8-way data parallel, 2048 blocks per core):
  - Host: diff = original - pred quantized to e4m3 (single rounding,
    ~3.5e-4 rel err on the scalar) -> halves HBM traffic (8.4 MB/core)
    and PE streaming vs shipping both operands.
  - Device: t = kron(I2, H64) @ x, one fp8 matmul per 512 moving cols
    (PSUM-bank cap). Back-to-back matmuls run at ~216 ns (LDWEIGHTS
    overlaps the previous matmul), so the PE is not the wall.
  - The wall is the PSUM drain: PSUM has one read port per engine,
    every reduce-capable op runs at 1 elem/cycle/partition, GPSIMD has
    no PSUM port, and the BIR verifier forbids two PSUM operands in
    one instruction. So the 65536 t-columns per core leave PSUM via
    VectorE tensor_reduce(abs+add) and ScalarE activation(Abs,
    accum_out) in parallel on different banks, 32 tiles each.
  - Each engine has its OWN double-buffered [128, 1024] PSUM pool
    (4 tiles = all 8 banks). Separate pools avoid head-of-line
    blocking on the in-order PE queue: a slow drain of one engine
    never blocks refills of the other engine's bank pair.
  - Per-tile partials land in one [128, 64] accumulator, DMA'd out
    once; the host does the final (tiny) summation in f64.
"""

from contextlib import ExitStack

import ml_dtypes
import numpy as np

import concourse.bass as bass
import concourse.tile as tile
from concourse import bacc, mybir
from concourse.bass_utils import run_bass_kernel_spmd

N_CORES = 8
N = 64                        # Hadamard block size
BLOCKS_TOTAL = 2 * 8192       # 16384 blocks of [64, 64]
BLOCKS_PER_CORE = BLOCKS_TOTAL // N_CORES   # 2048
GROUP_COLS = 1024             # psum tile free size (2 banks)
GROUPS = 64                   # psum tiles per core
GROUPS_PER_TILE = 8           # SBUF x-tile holds 8 groups (8 KiB/part)
ND = 33                       # tiles drained by VectorE
NA = GROUPS - ND              # tiles drained by ScalarE (1397 ns each)
TILES = GROUPS // GROUPS_PER_TILE           # 8
MM_N = 512                    # matmul moving cols (one PSUM bank out)

F32 = mybir.dt.float32
IN_DT = mybir.dt.float8e4
IN_NP = ml_dtypes.float8_e4m3


def _hadamard(n: int) -> np.ndarray:
    H = np.array([[1.0]], dtype=np.float32)
    while H.shape[0] < n:
        H = np.block([[H, H], [H, -H]])
    return H.astype(np.float32)


def _weights() -> np.ndarray:
    # lhsT for out = W @ rhs is W.T; kron(I2, H64) is symmetric.
    return np.kron(np.eye(2, dtype=np.float32),
                   _hadamard(N)).astype(IN_NP)  # [128, 128]


def _build_program() -> bacc.Bacc:
    nc = bacc.Bacc("TRN2", target_bir_lowering=False, debug=False,
                   num_devices=N_CORES)
    x = nc.dram_tensor("x", [TILES, 128, GROUPS_PER_TILE * GROUP_COLS],
                       IN_DT, kind="ExternalInput").ap()
    w = nc.dram_tensor("w", [128, 128], IN_DT, kind="ExternalInput").ap()
    out = nc.dram_tensor("out", [128, GROUPS], F32,
                         kind="ExternalOutput").ap()

    with tile.TileContext(nc) as tc, ExitStack() as ctx:
        wpool = ctx.enter_context(tc.tile_pool(name="w", bufs=1))
        xpool = ctx.enter_context(tc.tile_pool(name="x", bufs=3))
        psumd = ctx.enter_context(tc.tile_pool(name="psumd", bufs=2,
                                               space="PSUM"))
        psuma = ctx.enter_context(tc.tile_pool(name="psuma", bufs=2,
                                               space="PSUM"))
        accpool = ctx.enter_context(tc.tile_pool(name="acc", bufs=1))
        scrpool = ctx.enter_context(tc.tile_pool(name="scr", bufs=2))

        wt = wpool.tile([128, 128], IN_DT)

        acc = accpool.tile([128, GROUPS], F32, tag="acc")

        # Bresenham interleave of ND VectorE / NA ScalarE drains
        sched = []
        nd, na, err = ND, NA, 0
        while nd or na:
            if na == 0 or (nd > 0 and err <= 0):
                sched.append("D"); nd -= 1; err += NA
            else:
                sched.append("A"); na -= 1; err -= ND
        gi = 0
        di = 0
        ai = 0
        for t in range(TILES):
            xt = xpool.tile([128, GROUPS_PER_TILE * GROUP_COLS], IN_DT)
            if t == 0:
                # First x chunk before w: the first matmul needs both and
                # the 128 KiB chunk is the long pole; escalating chunk
                # sizes keep descriptor generation off the critical path.
                nc.sync.dma_start(xt[:, 0:1024], x[t, :, 0:1024])
                nc.sync.dma_start(wt[:], w[:])
                nc.sync.dma_start(xt[:, 1024:4096], x[t, :, 1024:4096])
                nc.sync.dma_start(xt[:, 4096:8192], x[t, :, 4096:8192])
            else:
                nc.sync.dma_start(xt[:], x[t])
            for g in range(GROUPS_PER_TILE):
                pt = (psumd if sched[gi] == "D" else psuma).tile(
                    [128, GROUP_COLS], F32)
                for q in range(GROUP_COLS // MM_N):
                    lo = g * GROUP_COLS + q * MM_N
                    nc.tensor.matmul(
                        pt[:, q * MM_N:(q + 1) * MM_N], wt[:],
                        xt[:, lo:lo + MM_N], start=True, stop=True)
                if sched[gi] == "D":
                    nc.vector.tensor_reduce(
                        acc[:, di:di + 1], pt[:],
                        axis=mybir.AxisListType.X, op=mybir.AluOpType.add,
                        apply_absolute_value=True)
                    di += 1
                else:
                    st = scrpool.tile([128, GROUP_COLS], F32)
                    nc.scalar.activation(
                        st[:], pt[:], mybir.ActivationFunctionType.Abs,
                        accum_out=acc[:, ND + ai:ND + ai + 1])
                    ai += 1
                gi += 1

        nc.sync.dma_start(out[:], acc[:])

    nc.compile()
    return nc


def _repack_core(shard: np.ndarray) -> np.ndarray:
    """[2048, 64, 64] e4m3 -> [TILES, 128, 8192]: partitions (m, j64),
    free (blk, k) within each SBUF tile."""
    v = shard.reshape(TILES, 2, 128, N, N)
    return v.transpose(0, 1, 3, 2, 4).reshape(
        TILES, 128, GROUPS_PER_TILE * GROUP_COLS)


_NC = None


def _get_program() -> bacc.Bacc:
    global _NC
    if _NC is None:
        _NC = _build_program()
    return _NC


def _run(original: np.ndarray, pred: np.ndarray, **spmd_kwargs):
    a = np.asarray(original, dtype=np.float32).reshape(BLOCKS_TOTAL, N, N)
    b = np.asarray(pred, dtype=np.float32).reshape(BLOCKS_TOTAL, N, N)
    diff = (a - b).astype(IN_NP)
    wnp = _weights()
    in_maps = []
    for i in range(N_CORES):
        sl = slice(i * BLOCKS_PER_CORE, (i + 1) * BLOCKS_PER_CORE)
        in_maps.append({"x": _repack_core(diff[sl]), "w": wnp})
    nc = _get_program()
    r = run_bass_kernel_spmd(nc, in_maps, list(range(N_CORES)),
                             **spmd_kwargs)
    total = 0.0
    for i in range(N_CORES):
        total += r.results[i]["out"].astype(np.float64).sum()
    return np.float32(total), r


def kernel(original: np.ndarray, pred: np.ndarray) -> np.ndarray:
    val, _ = _run(original, pred)
    return np.array(val, dtype=np.float32)
